# revision 11
# baseline (speedup 1.0000x reference)
"""GCLSTM (Chebyshev K=3 graph-conv LSTM gates) forward on 8 Trainium2 NeuronCores.

Math (derived from the reference model): the scan carry is unused and H/C start
at zero inside each step, so the output depends only on the LAST timestep and
every _cheb(H, ...) term reduces to its bias. What remains per output row i:

    deg[i]  = sum_{e: row[e]=i} w[e]
    dis     = deg > 0 ? 1/sqrt(max(deg, 1e-30)) : 0
    Y       = dis * X                      (host-precomputed)
    U1      = S(Y)       where  S(Z)[i] = sum_{e: row[e]=i} w[e] * Z[col[e]]
    Tx1     = -dis * U1
    U2      = S(dis^2 * U1)
    Tx2     = 2 * dis * U2 - X
    G_g     = X@(W[g,0]-W[g,2]) + Tx1@W[g,1] + (2*dis*U2)@W[g,2] + bias_g
    I = sigmoid(G_i); Tt = tanh(G_c); C = I*Tt
    O = sigmoid(G_o + wc[2]*C);  out = relu(O * tanh(C))

Sharding: nodes are 1-D partitioned across the 8 cores. SpMM1's gather table
(Y, host-computed) ships as a replicated input — no collective. SpMM2's table
(dis^2*U1) is exchanged via TWO AllGathers split by row-block range so the
first one overlaps SpMM1's tail and the second hides under SpMM2's first-half
gather calls. The small 128x128 gate weights are replicated.

Per-edge scatter-add = dense matmul against a host-built fp8 one-hot
"staircase" matrix with edges as the contraction dim; per-edge gathers use the
SWDGE dma_gather custom instruction (int16 indices; tables split in two
block-range halves to stay under the 32768-row index limit).
"""

import numpy as np
import ml_dtypes

P = 128
NCORES = 8
EDGE_NP = np.float16           # gather-table dtype
MT_NP = ml_dtypes.float8_e4m3  # one-hot scatter-matrix dtype
SWDGE_SCRATCH = 16384          # descriptor-ring carveout (ring limit 1024 descs)
CALL_G = 8                     # groups per dma_gather call (1024 idxs = ring limit)

# ----------------------------------------------------------------------------
# Host-side sharding / bucketing
# ----------------------------------------------------------------------------


def _preprocess(X, row, col, w):
    """Bucket edges by (owner core, col-block half, row block)."""
    N, F = X.shape
    assert F == P
    R = -(-N // NCORES)              # rows owned per core
    RB = -(-R // P)                  # 128-row blocks per core
    R_PAD = RB * P
    HB0 = (RB + 1) // 2              # blocks in half 0
    HB1 = RB - HB0
    NF0 = NCORES * HB0 * P           # rows of half-0 gather table
    NF1 = NCORES * HB1 * P
    assert NF0 <= 32768 and NF1 <= 32768, "int16 gather index limit"

    core = (row // R).astype(np.int64)
    lrow = (row - core * R).astype(np.int64)          # 0..R-1
    colc = col // R
    lcol = col - colc * R                             # 0..R-1 within owner
    cb = lcol // P                                    # col block within owner
    within = lcol - cb * P

    blk = lrow // P                                   # row block 0..RB-1
    half = (cb >= HB0).astype(np.int64)
    gidx = np.where(half == 0,
                    colc * (HB0 * P) + cb * P + within,
                    colc * (HB1 * P) + (cb - HB0) * P + within).astype(np.int64)
    key = half * RB + blk                             # half-major order

    cnt = np.zeros((NCORES, RB, 2), np.int64)
    np.add.at(cnt, (core, blk, half), 1)
    G = np.maximum(1, -(-cnt.max(axis=0) // P))       # [RB, 2]
    Lseg = np.ascontiguousarray(G.T) * P              # [2, RB] padded edges
    seg_start = np.concatenate([[0], np.cumsum(Lseg.ravel())])[:-1].reshape(2, RB)
    TOT = int(Lseg.sum())                             # padded edges per core
    TG = TOT // P                                     # total groups per core

    # host-side degree -> dis, Y = dis*X
    deg = np.zeros(N, np.float64)
    np.add.at(deg, row, w.astype(np.float64))
    dis = np.where(deg > 0, 1.0 / np.sqrt(np.maximum(deg, 1e-30)), 0.0).astype(np.float32)
    Ypad = np.zeros((NCORES, R_PAD, P), EDGE_NP)
    dis_pad = np.zeros((NCORES, R_PAD), np.float32)
    for c in range(NCORES):
        lo, hi = c * R, min((c + 1) * R, N)
        dis_pad[c, : hi - lo] = dis[lo:hi]
        Ypad[c, : hi - lo] = (dis[lo:hi, None] * X[lo:hi]).astype(EDGE_NP)
    yh0 = np.ascontiguousarray(Ypad[:, :HB0 * P].reshape(NF0, P))
    yh1 = np.ascontiguousarray(Ypad[:, HB0 * P:].reshape(NF1, P))

    in_maps = []
    for c in range(NCORES):
        sel = core == c
        lr_c = lrow[sel]
        gi_c = gidx[sel]
        w_c = w[sel]
        k_c = key[sel]

        order = np.argsort(k_c, kind="stable")
        lr_s, gi_s, w_s, k_s = lr_c[order], gi_c[order], w_c[order], k_c[order]
        cseg = np.bincount(k_s, minlength=2 * RB)
        within_e = np.arange(len(k_s)) - np.repeat(
            np.concatenate([[0], np.cumsum(cseg)])[:-1], cseg
        )
        pos = seg_start.ravel()[k_s] + within_e

        gi_arr = np.zeros(TOT, np.int64)
        w_arr = np.zeros(TOT, np.float32)
        lr_arr = np.zeros(TOT, np.int64)
        gi_arr[pos] = gi_s
        w_arr[pos] = w_s
        lr_arr[pos] = lr_s - (lr_s // P) * P

        idx16 = gi_arr.reshape(-1, 16).T              # [16, TOT/16]
        idx_all = np.tile(idx16, (8, 1)).astype(np.int16)
        mt_all = np.zeros((P, TG * P), MT_NP)
        e = np.arange(TOT)
        mt_all[e % P, (e // P) * P + lr_arr] = w_arr.astype(MT_NP)

        lo, hi = c * R, min((c + 1) * R, N)
        xl = np.zeros((R_PAD, P), np.float32)
        xl[: hi - lo] = X[lo:hi]
        dd = dis_pad[c].reshape(RB, P).T              # [P, RB]
        scal = np.concatenate([-dd, dd * dd, 2.0 * dd], axis=1)  # [P, 3*RB]

        in_maps.append(dict(idx_all=idx_all, mt_all=mt_all, x_loc=xl,
                            xt_loc=np.ascontiguousarray(xl.T),
                            yh0=yh0, yh1=yh1,
                            scal=np.ascontiguousarray(scal)))

    cfg = dict(N=N, R=R, RB=RB, R_PAD=R_PAD, HB0=HB0, HB1=HB1,
               NF0=NF0, NF1=NF1, TG=TG, G=G, seg_start=seg_start)
    return in_maps, cfg


# ----------------------------------------------------------------------------
# Device kernel
# ----------------------------------------------------------------------------


def _build(cfg):
    import concourse.bacc as bacc
    import concourse.mybir as mybir
    import concourse.tile as tile
    from concourse.masks import make_identity

    RB, TG = cfg["RB"], cfg["TG"]
    R_PAD, HB0, HB1 = cfg["R_PAD"], cfg["HB0"], cfg["HB1"]
    NF0, NF1 = cfg["NF0"], cfg["NF1"]
    G = cfg["G"]
    f32 = mybir.dt.float32
    f16 = mybir.dt.float16
    f8 = mybir.dt.float8e4
    Alu = mybir.AluOpType
    Act = mybir.ActivationFunctionType
    GATES = (0, 2, 3)  # i, c, o

    nc = bacc.Bacc("TRN2", target_bir_lowering=False, debug=False,
                   num_devices=NCORES, num_swdge_queues=4,
                   dynamic_dma_scratch_size=SWDGE_SCRATCH)

    x_loc = nc.dram_tensor("x_loc", [R_PAD, P], f32, kind="ExternalInput")
    xt_loc = nc.dram_tensor("xt_loc", [P, R_PAD], f32, kind="ExternalInput")
    yh0_t = nc.dram_tensor("yh0", [NF0, P], f16, kind="ExternalInput")
    yh1_t = nc.dram_tensor("yh1", [NF1, P], f16, kind="ExternalInput")
    scal_t = nc.dram_tensor("scal", [P, 3 * RB], f32, kind="ExternalInput")
    idx_all = nc.dram_tensor("idx_all", [P, TG * 8], mybir.dt.int16, kind="ExternalInput")
    mt_all = nc.dram_tensor("mt_all", [P, TG * P], f8, kind="ExternalInput")
    wx_t = nc.dram_tensor("wx_t", [4, 3, P, P], f32, kind="ExternalInput")
    bsum_t = nc.dram_tensor("bsum_t", [1, 4 * P], f32, kind="ExternalInput")
    wc_t = nc.dram_tensor("wc_t", [1, 3 * P], f32, kind="ExternalInput")
    out_loc = nc.dram_tensor("out_loc", [R_PAD, P], f16, kind="ExternalOutput")

    out_r = out_loc.rearrange("(b p) f -> p b f", p=P)

    cumG = np.concatenate([np.zeros((1, 2), np.int64),
                           np.cumsum(G, axis=0)], axis=0)   # [RB+1, 2]
    GSEG = [int(G[:, s].sum()) for s in range(2)]
    SEGG0 = [0, GSEG[0]]

    with tile.TileContext(nc) as tc:
        with (
            tc.tile_pool(name="const", bufs=1) as const,
            tc.tile_pool(name="pers", bufs=1) as pers,
            tc.tile_pool(name="work", bufs=3) as work,
            tc.tile_pool(name="vpool", bufs=12) as vpool,
            tc.tile_pool(name="mtpool", bufs=6) as mtpool,
            tc.tile_pool(name="ppool", bufs=3, space="PSUM") as ppool,
            tc.tile_pool(name="tpsum", bufs=2, space="PSUM") as tpsum,
            tc.tile_pool(name="gpsum", bufs=3, space="PSUM") as gpsum,
            tc.tile_pool(name="dram", bufs=1, space="DRAM") as dram,
        ):
            # ---------------- constants ----------------
            idx_sbs = []
            for s_ in range(2):
                lo = (SEGG0[s_]) * 8
                hi = (SEGG0[s_] + GSEG[s_]) * 8
                t = pers.tile([P, hi - lo], mybir.dt.int16, tag=f"idx{s_}")
                nc.sync.dma_start(out=t[:], in_=idx_all[:, lo:hi])
                idx_sbs.append(t)
            ident = const.tile([P, P], f32)
            make_identity(nc, ident[:])
            ones1 = const.tile([1, P], f32)
            nc.vector.memset(ones1[:], 1.0)

            wcat = {}
            for k in (0, 1, 2):
                t = const.tile([P, 3 * P], f32, tag=f"wcat{k}")
                for i, g in enumerate(GATES):
                    nc.sync.dma_start(out=t[:, i * P:(i + 1) * P], in_=wx_t[g, k])
                wcat[k] = t
            # fold W0 - W2 into wcat[0]
            nc.vector.tensor_tensor(out=wcat[0][:], in0=wcat[0][:],
                                    in1=wcat[2][:], op=Alu.subtract)

            bias_sb = const.tile([1, 4 * P], f32)
            nc.sync.dma_start(out=bias_sb[:], in_=bsum_t[:])
            bias_rep = {}
            wc_sb = const.tile([1, 3 * P], f32)
            nc.sync.dma_start(out=wc_sb[:], in_=wc_t[:])
            wc2_ps = tpsum.tile([P, P], f32, tag="tp")
            nc.tensor.matmul(out=wc2_ps[:], lhsT=ones1[:],
                             rhs=wc_sb[:, 2 * P:3 * P], start=True, stop=True)
            wc2_rep = const.tile([P, P], f32)
            nc.scalar.copy(out=wc2_rep[:], in_=wc2_ps[:])
            for g in (0, 2, 3):
                bp = tpsum.tile([P, P], f32, tag="tp", space="PSUM")
                nc.tensor.matmul(out=bp[:], lhsT=ones1[:],
                                 rhs=bias_sb[:, g * P:(g + 1) * P],
                                 start=True, stop=True)
                br = const.tile([P, P], f32, tag=f"br{g}")
                nc.scalar.copy(out=br[:], in_=bp[:])
                bias_rep[g] = br

            scal_sb = const.tile([P, 3 * RB], f32)
            nc.sync.dma_start(out=scal_sb[:], in_=scal_t[:])
            ndis = scal_sb[:, 0:RB]
            dis2 = scal_sb[:, RB:2 * RB]
            dis2x = scal_sb[:, 2 * RB:3 * RB]

            xt_sb = pers.tile([P, RB, P], f32, tag="xt")
            nc.sync.dma_start(
                out=xt_sb[:],
                in_=xt_loc.rearrange("p (b q) -> p b q", b=RB))

            # SpMM2 exchange buffers (two block-range halves)
            y2ag0 = dram.tile([HB0 * P, P], f16)
            y2ag0_r = y2ag0[:].rearrange("(b p) f -> p b f", p=P)
            y2ag1 = dram.tile([HB1 * P, P], f16)
            y2ag1_r = y2ag1[:].rearrange("(b p) f -> p b f", p=P)
            y2f0 = dram.tile([NF0, P], f16, addr_space="Shared")
            y2f1 = dram.tile([NF1, P], f16, addr_space="Shared")

            qctr = [0]

            def spmm(srcs, consume, ph):
                for s in range(2):
                    src_ap = srcs[s]
                    hoff = SEGG0[s]
                    nh = GSEG[s]
                    vt = {}
                    emitted = [-1]

                    def ensure_call(k, s=s, hoff=hoff, nh=nh, vt=vt,
                                    emitted=emitted, src_ap=src_ap):
                        while emitted[0] < k:
                            kk = emitted[0] + 1
                            gc = min(CALL_G, nh - kk * CALL_G)
                            eoff = (kk * CALL_G) * P
                            v = vpool.tile([P, CALL_G, P], f16, tag="v",
                                           name=f"v{ph}_{s}_{kk}")
                            nc.gpsimd.dma_gather(
                                out_ap=v[:, :gc, :],
                                in_ap=src_ap,
                                idxs_ap=idx_sbs[s][:, eoff // 16:(eoff + gc * P) // 16],
                                num_idxs=gc * P, num_idxs_reg=gc * P,
                                elem_size=P, queue_num=qctr[0] % 4)
                            qctr[0] += 1
                            vt[kk] = v
                            vt.pop(kk - 16, None)
                            emitted[0] = kk
                    for b in range(RB):
                        s_b, e_b = int(cumG[b, s]), int(cumG[b + 1, s])
                        gs = e_b - s_b
                        goff = hoff + s_b
                        mt = mtpool.tile([P, int(G.max()) * P], f8, tag="mt")
                        nc.sync.dma_start(
                            out=mt[:, :gs * P],
                            in_=mt_all[:, goff * P:(goff + gs) * P])
                        ps = ppool.tile([P, P], f32, tag="u", name=f"ps{ph}_{s}_{b}")
                        for gl_ in range(s_b, e_b):
                            k = gl_ // CALL_G
                            ensure_call(min(k + 5, (nh - 1) // CALL_G))
                            nc.tensor.matmul(
                                out=ps[:], lhsT=mt[:, (gl_ - s_b) * P:(gl_ - s_b + 1) * P],
                                rhs=vt[k][:, gl_ % CALL_G, :],
                                start=(gl_ == s_b), stop=(gl_ == e_b - 1))
                        consume(b, ps, s)

            # ---------------- SpMM 1: U1 = S(Y) ----------------
            u1_sb = pers.tile([P, RB, P], f32, tag="u1")
            y2_sb = pers.tile([P, RB, P], f16, tag="y2")
            CH0 = (0, HB0 // 2, HB0)
            CH1 = (HB0, HB0 + (RB - HB0) // 2, RB)

            def consume1(b, ps, s):
                if s == 0:
                    nc.scalar.copy(out=u1_sb[:, b, :], in_=ps[:])
                    return
                usum = work.tile([P, P], f32, tag="us1")
                nc.vector.tensor_tensor(out=usum[:], in0=u1_sb[:, b, :],
                                        in1=ps[:], op=Alu.add)
                # y2[b] = dis^2 * U1[b] staged in SBUF, flushed in chunks
                nc.vector.tensor_tensor(out=y2_sb[:, b, :], in0=usum[:],
                                        in1=dis2[:, b:b + 1].to_broadcast([P, P]),
                                        op=Alu.mult)
                if b + 1 in CH0[1:]:
                    i = CH0[1:].index(b + 1)
                    lo = CH0[i]
                    nc.sync.dma_start(out=y2ag0_r[:, lo:b + 1, :],
                                      in_=y2_sb[:, lo:b + 1, :])
                elif b + 1 in CH1[1:]:
                    i = CH1[1:].index(b + 1)
                    lo = CH1[i]
                    nc.sync.dma_start(out=y2ag1_r[:, lo - HB0:b + 1 - HB0, :],
                                      in_=y2_sb[:, lo:b + 1, :])
                # A = -dis * U1 (consumed by the gates in SpMM2)
                nc.vector.tensor_tensor(out=u1_sb[:, b, :], in0=usum[:],
                                        in1=ndis[:, b:b + 1].to_broadcast([P, P]),
                                        op=Alu.mult)
                if b == HB0 - 1:
                    nc.gpsimd.collective_compute(
                        "AllGather", Alu.bypass,
                        replica_groups=[list(range(NCORES))],
                        ins=[y2ag0.opt()], outs=[y2f0.opt()])
                if b == RB - 1:
                    nc.gpsimd.collective_compute(
                        "AllGather", Alu.bypass,
                        replica_groups=[list(range(NCORES))],
                        ins=[y2ag1.opt()], outs=[y2f1.opt()])

            spmm((yh0_t[:, :], yh1_t[:, :]), consume1, 0)

            # ---------------- SpMM 2 + gates, fused per block --------------
            u2_sb = pers.tile([P, RB, P], f32, tag="u2")

            def consume2(b, ps2, s):
                if s == 0:
                    nc.scalar.copy(out=u2_sb[:, b, :], in_=ps2[:])
                    return
                usum = work.tile([P, P], f32, tag="usum")
                nc.vector.tensor_tensor(out=usum[:], in0=u2_sb[:, b, :],
                                        in1=ps2[:], op=Alu.add)
                bt_sb = work.tile([P, P], f32, tag="bt")
                nc.vector.tensor_tensor(out=bt_sb[:], in0=usum[:],
                                        in1=dis2x[:, b:b + 1].to_broadcast([P, P]),
                                        op=Alu.mult)
                xt = xt_sb[:, b, :]
                tmats = []
                for srcp, tag in ((u1_sb[:, b, :], "at"), (bt_sb[:], "bt2")):
                    tp = tpsum.tile([P, P], f32, tag="tp", space="PSUM")
                    nc.tensor.transpose(out=tp[:], in_=srcp, identity=ident[:])
                    ts = work.tile([P, P], f32, tag=tag)
                    nc.scalar.copy(out=ts[:], in_=tp[:])
                    tmats.append(ts)
                at, btm = tmats
                pg = gpsum.tile([P, 3 * P], f32, tag="g", space="PSUM")
                nc.tensor.matmul(out=pg[:], lhsT=xt, rhs=wcat[0][:],
                                 start=True, stop=False)
                nc.tensor.matmul(out=pg[:], lhsT=at[:], rhs=wcat[1][:],
                                 start=False, stop=False)
                nc.tensor.matmul(out=pg[:], lhsT=btm[:], rhs=wcat[2][:],
                                 start=False, stop=True)
                gate_ps = {0: pg[:, 0:P], 2: pg[:, P:2 * P], 3: pg[:, 2 * P:3 * P]}
                gi = work.tile([P, P], f32, tag="gi")
                nc.vector.tensor_tensor(out=gi[:], in0=gate_ps[0],
                                        in1=bias_rep[0][:], op=Alu.add)
                i_t = work.tile([P, P], f32, tag="i")
                nc.scalar.activation(out=i_t[:], in_=gi[:], func=Act.Sigmoid)
                gc_ = work.tile([P, P], f32, tag="gc")
                nc.vector.tensor_tensor(out=gc_[:], in0=gate_ps[2],
                                        in1=bias_rep[2][:], op=Alu.add)
                tt_t = work.tile([P, P], f32, tag="tt")
                nc.scalar.activation(out=tt_t[:], in_=gc_[:], func=Act.Tanh)
                c_t = work.tile([P, P], f32, tag="c")
                nc.vector.tensor_tensor(out=c_t[:], in0=i_t[:], in1=tt_t[:],
                                        op=Alu.mult)
                wcc = work.tile([P, P], f32, tag="wcc")
                nc.vector.tensor_tensor(out=wcc[:], in0=c_t[:], in1=wc2_rep[:],
                                        op=Alu.mult)
                wcb = work.tile([P, P], f32, tag="wcb")
                nc.vector.tensor_tensor(out=wcb[:], in0=wcc[:], in1=bias_rep[3][:],
                                        op=Alu.add)
                oin = work.tile([P, P], f32, tag="oin")
                nc.vector.tensor_tensor(out=oin[:], in0=gate_ps[3], in1=wcb[:],
                                        op=Alu.add)
                o_t = work.tile([P, P], f32, tag="o")
                nc.scalar.activation(out=o_t[:], in_=oin[:], func=Act.Sigmoid)
                tc_t = work.tile([P, P], f32, tag="tc")
                nc.scalar.activation(out=tc_t[:], in_=c_t[:], func=Act.Tanh)
                h_t = work.tile([P, P], f32, tag="h")
                nc.vector.tensor_tensor(out=h_t[:], in0=o_t[:], in1=tc_t[:],
                                        op=Alu.mult)
                res = work.tile([P, P], f16, tag="res")
                nc.scalar.activation(out=res[:], in_=h_t[:], func=Act.Relu)
                nc.sync.dma_start(out=out_r[:, b, :], in_=res[:])

            spmm((y2f0[:], y2f1[:]), consume2, 1)

    nc.compile()
    return nc


# ----------------------------------------------------------------------------
# Entry point
# ----------------------------------------------------------------------------

_CACHE = {}


def _get_built(cfg_key, cfg):
    if cfg_key not in _CACHE:
        _CACHE[cfg_key] = _build(cfg)
    return _CACHE[cfg_key]


def _make_in_maps(inputs):
    node_feats = np.asarray(inputs["node_feats"])
    edge_feats = np.asarray(inputs["edge_feats"], np.float32)
    edge_index = np.asarray(inputs["edge_index"])
    t = node_feats.shape[0] - 1
    X = np.asarray(node_feats[t], np.float32)
    row = np.asarray(edge_index[t, 0], np.int64)
    col = np.asarray(edge_index[t, 1], np.int64)
    w = np.asarray(edge_feats[t], np.float32)

    in_maps, cfg = _preprocess(X, row, col, w)

    Wx = np.asarray(inputs["Wx"], np.float32)
    bsum = (np.asarray(inputs["bx"], np.float32)
            + np.asarray(inputs["bh"], np.float32)
            + np.asarray(inputs["bg"], np.float32)).reshape(1, -1)
    wc = np.asarray(inputs["wc"], np.float32).reshape(1, -1)
    for m in in_maps:
        m["wx_t"] = Wx
        m["bsum_t"] = bsum
        m["wc_t"] = wc
    return in_maps, cfg


def _run(inputs, trace=False):
    from concourse.bass_utils import run_bass_kernel_spmd

    in_maps, cfg = _make_in_maps(inputs)
    key = (cfg["N"], cfg["RB"], cfg["TG"],
           tuple(cfg["G"].ravel().tolist()))
    nc = _get_built(key, cfg)
    res = run_bass_kernel_spmd(nc, in_maps, core_ids=list(range(NCORES)),
                               trace=trace)
    N, R, R_PAD = cfg["N"], cfg["R"], cfg["R_PAD"]
    out = np.empty((N, P), np.float32)
    for c in range(NCORES):
        lo, hi = c * R, min((c + 1) * R, N)
        out[lo:hi] = res.results[c]["out_loc"][: hi - lo].astype(np.float32)
    return out, res.exec_time_ns


def kernel(**inputs) -> np.ndarray:
    out, _ = _run(inputs, trace=False)
    return out


# revision 13
# speedup vs baseline: 1.0182x; 1.0182x over previous
"""GCLSTM (Chebyshev K=3 graph-conv LSTM gates) forward on 8 Trainium2 NeuronCores.

Math (derived from the reference model): the scan carry is unused and H/C start
at zero inside each step, so the output depends only on the LAST timestep and
every _cheb(H, ...) term reduces to its bias. What remains per output row i:

    deg[i]  = sum_{e: row[e]=i} w[e]
    dis     = deg > 0 ? 1/sqrt(max(deg, 1e-30)) : 0
    Y       = dis * X                      (host-precomputed)
    U1      = S(Y)       where  S(Z)[i] = sum_{e: row[e]=i} w[e] * Z[col[e]]
    Tx1     = -dis * U1
    U2      = S(dis^2 * U1)
    Tx2     = 2 * dis * U2 - X
    G_g     = X@(W[g,0]-W[g,2]) + Tx1@W[g,1] + (2*dis*U2)@W[g,2] + bias_g
    I = sigmoid(G_i); Tt = tanh(G_c); C = I*Tt
    O = sigmoid(G_o + wc[2]*C);  out = relu(O * tanh(C))

Sharding: nodes are 1-D partitioned across the 8 cores. SpMM1's gather table
(Y, host-computed) ships as a replicated input — no collective. SpMM2's table
(dis^2*U1) is exchanged via TWO AllGathers split by row-block range so the
first one overlaps SpMM1's tail and the second hides under SpMM2's first-half
gather calls. The small 128x128 gate weights are replicated.

Per-edge scatter-add = dense matmul against a host-built fp8 one-hot
"staircase" matrix with edges as the contraction dim; per-edge gathers use the
SWDGE dma_gather custom instruction (int16 indices; tables split in two
block-range halves to stay under the 32768-row index limit).
"""

import numpy as np
import ml_dtypes

P = 128
NCORES = 8
EDGE_NP = np.float16           # gather-table dtype
MT_NP = ml_dtypes.float8_e4m3  # one-hot scatter-matrix dtype
SWDGE_SCRATCH = 16384          # descriptor-ring carveout (ring limit 1024 descs)
CALL_G = 8                     # groups per dma_gather call (1024 idxs = ring limit)

# ----------------------------------------------------------------------------
# Host-side sharding / bucketing
# ----------------------------------------------------------------------------


def _preprocess(X, row, col, w):
    """Bucket edges by (owner core, col-block half, row block)."""
    N, F = X.shape
    assert F == P
    R = -(-N // NCORES)              # rows owned per core
    RB = -(-R // P)                  # 128-row blocks per core
    R_PAD = RB * P
    HB0 = (RB + 1) // 2              # blocks in half 0
    HB1 = RB - HB0
    NF0 = NCORES * HB0 * P           # rows of half-0 gather table
    NF1 = NCORES * HB1 * P
    assert NF0 <= 32768 and NF1 <= 32768, "int16 gather index limit"

    core = (row // R).astype(np.int64)
    lrow = (row - core * R).astype(np.int64)          # 0..R-1
    colc = col // R
    lcol = col - colc * R                             # 0..R-1 within owner
    cb = lcol // P                                    # col block within owner
    within = lcol - cb * P

    blk = lrow // P                                   # row block 0..RB-1
    half = (cb >= HB0).astype(np.int64)
    gidx = np.where(half == 0,
                    colc * (HB0 * P) + cb * P + within,
                    colc * (HB1 * P) + (cb - HB0) * P + within).astype(np.int64)
    key = half * RB + blk                             # half-major order

    cnt = np.zeros((NCORES, RB, 2), np.int64)
    np.add.at(cnt, (core, blk, half), 1)
    G = np.maximum(1, -(-cnt.max(axis=0) // P))       # [RB, 2]
    Lseg = np.ascontiguousarray(G.T) * P              # [2, RB] padded edges
    seg_start = np.concatenate([[0], np.cumsum(Lseg.ravel())])[:-1].reshape(2, RB)
    TOT = int(Lseg.sum())                             # padded edges per core
    TG = TOT // P                                     # total groups per core

    # host-side degree -> dis, Y = dis*X
    deg = np.zeros(N, np.float64)
    np.add.at(deg, row, w.astype(np.float64))
    dis = np.where(deg > 0, 1.0 / np.sqrt(np.maximum(deg, 1e-30)), 0.0).astype(np.float32)
    Ypad = np.zeros((NCORES, R_PAD, P), EDGE_NP)
    dis_pad = np.zeros((NCORES, R_PAD), np.float32)
    for c in range(NCORES):
        lo, hi = c * R, min((c + 1) * R, N)
        dis_pad[c, : hi - lo] = dis[lo:hi]
        Ypad[c, : hi - lo] = (dis[lo:hi, None] * X[lo:hi]).astype(EDGE_NP)
    yh0 = np.ascontiguousarray(Ypad[:, :HB0 * P].reshape(NF0, P))
    yh1 = np.ascontiguousarray(Ypad[:, HB0 * P:].reshape(NF1, P))

    in_maps = []
    for c in range(NCORES):
        sel = core == c
        lr_c = lrow[sel]
        gi_c = gidx[sel]
        w_c = w[sel]
        k_c = key[sel]

        order = np.argsort(k_c, kind="stable")
        lr_s, gi_s, w_s, k_s = lr_c[order], gi_c[order], w_c[order], k_c[order]
        cseg = np.bincount(k_s, minlength=2 * RB)
        within_e = np.arange(len(k_s)) - np.repeat(
            np.concatenate([[0], np.cumsum(cseg)])[:-1], cseg
        )
        pos = seg_start.ravel()[k_s] + within_e

        gi_arr = np.zeros(TOT, np.int64)
        w_arr = np.zeros(TOT, np.float32)
        lr_arr = np.zeros(TOT, np.int64)
        gi_arr[pos] = gi_s
        w_arr[pos] = w_s
        lr_arr[pos] = lr_s - (lr_s // P) * P

        idx16 = gi_arr.reshape(-1, 16).T              # [16, TOT/16]
        idx_all = np.tile(idx16, (8, 1)).astype(np.int16)
        mt_all = np.zeros((P, TG * P), MT_NP)
        e = np.arange(TOT)
        mt_all[e % P, (e // P) * P + lr_arr] = w_arr.astype(MT_NP)

        lo, hi = c * R, min((c + 1) * R, N)
        xl = np.zeros((R_PAD, P), np.float32)
        xl[: hi - lo] = X[lo:hi]
        dd = dis_pad[c].reshape(RB, P).T              # [P, RB]
        scal = np.concatenate([-dd, dd * dd, 2.0 * dd], axis=1)  # [P, 3*RB]

        in_maps.append(dict(idx_all=idx_all, mt_all=mt_all, x_loc=xl,
                            xt_loc=np.ascontiguousarray(xl.T),
                            yh0=yh0, yh1=yh1,
                            scal=np.ascontiguousarray(scal)))

    cfg = dict(N=N, R=R, RB=RB, R_PAD=R_PAD, HB0=HB0, HB1=HB1,
               NF0=NF0, NF1=NF1, TG=TG, G=G, seg_start=seg_start)
    return in_maps, cfg


# ----------------------------------------------------------------------------
# Device kernel
# ----------------------------------------------------------------------------


def _build(cfg):
    import concourse.bacc as bacc
    import concourse.mybir as mybir
    import concourse.tile as tile
    from concourse.masks import make_identity

    RB, TG = cfg["RB"], cfg["TG"]
    R_PAD, HB0, HB1 = cfg["R_PAD"], cfg["HB0"], cfg["HB1"]
    NF0, NF1 = cfg["NF0"], cfg["NF1"]
    G = cfg["G"]
    f32 = mybir.dt.float32
    f16 = mybir.dt.float16
    f8 = mybir.dt.float8e4
    Alu = mybir.AluOpType
    Act = mybir.ActivationFunctionType
    GATES = (0, 2, 3)  # i, c, o

    nc = bacc.Bacc("TRN2", target_bir_lowering=False, debug=False,
                   num_devices=NCORES, num_swdge_queues=4,
                   dynamic_dma_scratch_size=SWDGE_SCRATCH)

    x_loc = nc.dram_tensor("x_loc", [R_PAD, P], f32, kind="ExternalInput")
    xt_loc = nc.dram_tensor("xt_loc", [P, R_PAD], f32, kind="ExternalInput")
    yh0_t = nc.dram_tensor("yh0", [NF0, P], f16, kind="ExternalInput")
    yh1_t = nc.dram_tensor("yh1", [NF1, P], f16, kind="ExternalInput")
    scal_t = nc.dram_tensor("scal", [P, 3 * RB], f32, kind="ExternalInput")
    idx_all = nc.dram_tensor("idx_all", [P, TG * 8], mybir.dt.int16, kind="ExternalInput")
    mt_all = nc.dram_tensor("mt_all", [P, TG * P], f8, kind="ExternalInput")
    wx_t = nc.dram_tensor("wx_t", [4, 3, P, P], f32, kind="ExternalInput")
    bsum_t = nc.dram_tensor("bsum_t", [1, 4 * P], f32, kind="ExternalInput")
    wc_t = nc.dram_tensor("wc_t", [1, 3 * P], f32, kind="ExternalInput")
    out_loc = nc.dram_tensor("out_loc", [R_PAD, P], f16, kind="ExternalOutput")

    out_r = out_loc.rearrange("(b p) f -> p b f", p=P)

    cumG = np.concatenate([np.zeros((1, 2), np.int64),
                           np.cumsum(G, axis=0)], axis=0)   # [RB+1, 2]
    GSEG = [int(G[:, s].sum()) for s in range(2)]
    SEGG0 = [0, GSEG[0]]

    with tile.TileContext(nc) as tc:
        with (
            tc.tile_pool(name="const", bufs=1) as const,
            tc.tile_pool(name="pers", bufs=1) as pers,
            tc.tile_pool(name="work", bufs=3) as work,
            tc.tile_pool(name="vpool", bufs=12) as vpool,
            tc.tile_pool(name="mtpool", bufs=8) as mtpool,
            tc.tile_pool(name="ppool", bufs=3, space="PSUM") as ppool,
            tc.tile_pool(name="tpsum", bufs=2, space="PSUM") as tpsum,
            tc.tile_pool(name="gpsum", bufs=3, space="PSUM") as gpsum,
            tc.tile_pool(name="dram", bufs=1, space="DRAM") as dram,
        ):
            # ---------------- constants ----------------
            idx_sbs = []
            for s_ in range(2):
                lo = (SEGG0[s_]) * 8
                hi = (SEGG0[s_] + GSEG[s_]) * 8
                t = pers.tile([P, hi - lo], mybir.dt.int16, tag=f"idx{s_}")
                nc.sync.dma_start(out=t[:], in_=idx_all[:, lo:hi])
                idx_sbs.append(t)
            ident = const.tile([P, P], f32)
            make_identity(nc, ident[:])
            ones1 = const.tile([1, P], f32)
            nc.vector.memset(ones1[:], 1.0)

            wcat = {}
            for k in (0, 1, 2):
                t = const.tile([P, 3 * P], f32, tag=f"wcat{k}")
                for i, g in enumerate(GATES):
                    nc.sync.dma_start(out=t[:, i * P:(i + 1) * P], in_=wx_t[g, k])
                wcat[k] = t
            # fold W0 - W2 into wcat[0]
            nc.vector.tensor_tensor(out=wcat[0][:], in0=wcat[0][:],
                                    in1=wcat[2][:], op=Alu.subtract)

            bias_sb = const.tile([1, 4 * P], f32)
            nc.sync.dma_start(out=bias_sb[:], in_=bsum_t[:])
            bias_rep = {}
            wc_sb = const.tile([1, 3 * P], f32)
            nc.sync.dma_start(out=wc_sb[:], in_=wc_t[:])
            wc2_ps = tpsum.tile([P, P], f32, tag="tp")
            nc.tensor.matmul(out=wc2_ps[:], lhsT=ones1[:],
                             rhs=wc_sb[:, 2 * P:3 * P], start=True, stop=True)
            wc2_rep = const.tile([P, P], f32)
            nc.scalar.copy(out=wc2_rep[:], in_=wc2_ps[:])
            for g in (0, 2, 3):
                bp = tpsum.tile([P, P], f32, tag="tp", space="PSUM")
                nc.tensor.matmul(out=bp[:], lhsT=ones1[:],
                                 rhs=bias_sb[:, g * P:(g + 1) * P],
                                 start=True, stop=True)
                br = const.tile([P, P], f32, tag=f"br{g}")
                nc.scalar.copy(out=br[:], in_=bp[:])
                bias_rep[g] = br

            scal_sb = const.tile([P, 3 * RB], f32)
            nc.sync.dma_start(out=scal_sb[:], in_=scal_t[:])
            ndis = scal_sb[:, 0:RB]
            dis2 = scal_sb[:, RB:2 * RB]
            dis2x = scal_sb[:, 2 * RB:3 * RB]

            xt_sb = pers.tile([P, RB, P], f32, tag="xt")
            nc.sync.dma_start(
                out=xt_sb[:],
                in_=xt_loc.rearrange("p (b q) -> p b q", b=RB))

            # SpMM2 exchange buffers (two block-range halves)
            y2ag0 = dram.tile([HB0 * P, P], f16)
            y2ag0_r = y2ag0[:].rearrange("(b p) f -> p b f", p=P)
            y2ag1 = dram.tile([HB1 * P, P], f16)
            y2ag1_r = y2ag1[:].rearrange("(b p) f -> p b f", p=P)
            y2f0 = dram.tile([NF0, P], f16, addr_space="Shared")
            y2f1 = dram.tile([NF1, P], f16, addr_space="Shared")

            qctr = [0]

            def spmm(srcs, consume, ph):
                for s in range(2):
                    src_ap = srcs[s]
                    hoff = SEGG0[s]
                    nh = GSEG[s]
                    vt = {}
                    emitted = [-1]

                    def ensure_call(k, s=s, hoff=hoff, nh=nh, vt=vt,
                                    emitted=emitted, src_ap=src_ap):
                        while emitted[0] < k:
                            kk = emitted[0] + 1
                            gc = min(CALL_G, nh - kk * CALL_G)
                            eoff = (kk * CALL_G) * P
                            v = vpool.tile([P, CALL_G, P], f16, tag="v",
                                           name=f"v{ph}_{s}_{kk}")
                            nc.gpsimd.dma_gather(
                                out_ap=v[:, :gc, :],
                                in_ap=src_ap,
                                idxs_ap=idx_sbs[s][:, eoff // 16:(eoff + gc * P) // 16],
                                num_idxs=gc * P, num_idxs_reg=gc * P,
                                elem_size=P, queue_num=qctr[0] % 4)
                            qctr[0] += 1
                            vt[kk] = v
                            vt.pop(kk - 16, None)
                            emitted[0] = kk
                    for b in range(RB):
                        s_b, e_b = int(cumG[b, s]), int(cumG[b + 1, s])
                        gs = e_b - s_b
                        goff = hoff + s_b
                        mt = mtpool.tile([P, int(G.max()) * P], f8, tag="mt")
                        nc.sync.dma_start(
                            out=mt[:, :gs * P],
                            in_=mt_all[:, goff * P:(goff + gs) * P])
                        ps = ppool.tile([P, P], f32, tag="u", name=f"ps{ph}_{s}_{b}")
                        for gl_ in range(s_b, e_b):
                            k = gl_ // CALL_G
                            ensure_call(min(k + 7, (nh - 1) // CALL_G))
                            nc.tensor.matmul(
                                out=ps[:], lhsT=mt[:, (gl_ - s_b) * P:(gl_ - s_b + 1) * P],
                                rhs=vt[k][:, gl_ % CALL_G, :],
                                start=(gl_ == s_b), stop=(gl_ == e_b - 1))
                        consume(b, ps, s)

            # ---------------- SpMM 1: U1 = S(Y) ----------------
            u1_sb = pers.tile([P, RB, P], f32, tag="u1")

            def consume1(b, ps, s):
                if s == 0:
                    nc.scalar.copy(out=u1_sb[:, b, :], in_=ps[:])
                    return
                usum = work.tile([P, P], f32, tag="us1")
                nc.vector.tensor_tensor(out=usum[:], in0=u1_sb[:, b, :],
                                        in1=ps[:], op=Alu.add)
                # y2[b] = dis^2 * U1[b] -> exchange buffer
                yt = work.tile([P, P], f16, tag="yt")
                nc.vector.tensor_tensor(out=yt[:], in0=usum[:],
                                        in1=dis2[:, b:b + 1].to_broadcast([P, P]),
                                        op=Alu.mult)
                if b < HB0:
                    nc.sync.dma_start(out=y2ag0_r[:, b, :], in_=yt[:])
                else:
                    nc.sync.dma_start(out=y2ag1_r[:, b - HB0, :], in_=yt[:])
                # A = -dis * U1 (consumed by the gates in SpMM2)
                nc.vector.tensor_tensor(out=u1_sb[:, b, :], in0=usum[:],
                                        in1=ndis[:, b:b + 1].to_broadcast([P, P]),
                                        op=Alu.mult)
                if b == HB0 - 1:
                    nc.gpsimd.collective_compute(
                        "AllGather", Alu.bypass,
                        replica_groups=[list(range(NCORES))],
                        ins=[y2ag0.opt()], outs=[y2f0.opt()])
                if b == RB - 1:
                    nc.gpsimd.collective_compute(
                        "AllGather", Alu.bypass,
                        replica_groups=[list(range(NCORES))],
                        ins=[y2ag1.opt()], outs=[y2f1.opt()])

            spmm((yh0_t[:, :], yh1_t[:, :]), consume1, 0)

            # ---------------- SpMM 2 + gates, fused per block --------------
            u2_sb = pers.tile([P, RB, P], f32, tag="u2")

            def consume2(b, ps2, s):
                if s == 0:
                    nc.scalar.copy(out=u2_sb[:, b, :], in_=ps2[:])
                    return
                usum = work.tile([P, P], f32, tag="usum")
                nc.vector.tensor_tensor(out=usum[:], in0=u2_sb[:, b, :],
                                        in1=ps2[:], op=Alu.add)
                bt_sb = work.tile([P, P], f32, tag="bt")
                nc.vector.tensor_tensor(out=bt_sb[:], in0=usum[:],
                                        in1=dis2x[:, b:b + 1].to_broadcast([P, P]),
                                        op=Alu.mult)
                xt = xt_sb[:, b, :]
                tmats = []
                for srcp, tag in ((u1_sb[:, b, :], "at"), (bt_sb[:], "bt2")):
                    tp = tpsum.tile([P, P], f32, tag="tp", space="PSUM")
                    nc.tensor.transpose(out=tp[:], in_=srcp, identity=ident[:])
                    ts = work.tile([P, P], f32, tag=tag)
                    nc.scalar.copy(out=ts[:], in_=tp[:])
                    tmats.append(ts)
                at, btm = tmats
                pg = gpsum.tile([P, 3 * P], f32, tag="g", space="PSUM")
                nc.tensor.matmul(out=pg[:], lhsT=xt, rhs=wcat[0][:],
                                 start=True, stop=False)
                nc.tensor.matmul(out=pg[:], lhsT=at[:], rhs=wcat[1][:],
                                 start=False, stop=False)
                nc.tensor.matmul(out=pg[:], lhsT=btm[:], rhs=wcat[2][:],
                                 start=False, stop=True)
                gate_ps = {0: pg[:, 0:P], 2: pg[:, P:2 * P], 3: pg[:, 2 * P:3 * P]}
                gi = work.tile([P, P], f32, tag="gi")
                nc.vector.tensor_tensor(out=gi[:], in0=gate_ps[0],
                                        in1=bias_rep[0][:], op=Alu.add)
                i_t = work.tile([P, P], f32, tag="i")
                nc.scalar.activation(out=i_t[:], in_=gi[:], func=Act.Sigmoid)
                gc_ = work.tile([P, P], f32, tag="gc")
                nc.vector.tensor_tensor(out=gc_[:], in0=gate_ps[2],
                                        in1=bias_rep[2][:], op=Alu.add)
                tt_t = work.tile([P, P], f32, tag="tt")
                nc.scalar.activation(out=tt_t[:], in_=gc_[:], func=Act.Tanh)
                c_t = work.tile([P, P], f32, tag="c")
                nc.vector.tensor_tensor(out=c_t[:], in0=i_t[:], in1=tt_t[:],
                                        op=Alu.mult)
                wcc = work.tile([P, P], f32, tag="wcc")
                nc.vector.tensor_tensor(out=wcc[:], in0=c_t[:], in1=wc2_rep[:],
                                        op=Alu.mult)
                wcb = work.tile([P, P], f32, tag="wcb")
                nc.vector.tensor_tensor(out=wcb[:], in0=wcc[:], in1=bias_rep[3][:],
                                        op=Alu.add)
                oin = work.tile([P, P], f32, tag="oin")
                nc.vector.tensor_tensor(out=oin[:], in0=gate_ps[3], in1=wcb[:],
                                        op=Alu.add)
                o_t = work.tile([P, P], f32, tag="o")
                nc.scalar.activation(out=o_t[:], in_=oin[:], func=Act.Sigmoid)
                tc_t = work.tile([P, P], f32, tag="tc")
                nc.scalar.activation(out=tc_t[:], in_=c_t[:], func=Act.Tanh)
                h_t = work.tile([P, P], f32, tag="h")
                nc.vector.tensor_tensor(out=h_t[:], in0=o_t[:], in1=tc_t[:],
                                        op=Alu.mult)
                res = work.tile([P, P], f16, tag="res")
                nc.scalar.activation(out=res[:], in_=h_t[:], func=Act.Relu)
                nc.sync.dma_start(out=out_r[:, b, :], in_=res[:])

            spmm((y2f0[:], y2f1[:]), consume2, 1)

    nc.compile()
    return nc


# ----------------------------------------------------------------------------
# Entry point
# ----------------------------------------------------------------------------

_CACHE = {}


def _get_built(cfg_key, cfg):
    if cfg_key not in _CACHE:
        _CACHE[cfg_key] = _build(cfg)
    return _CACHE[cfg_key]


def _make_in_maps(inputs):
    node_feats = np.asarray(inputs["node_feats"])
    edge_feats = np.asarray(inputs["edge_feats"], np.float32)
    edge_index = np.asarray(inputs["edge_index"])
    t = node_feats.shape[0] - 1
    X = np.asarray(node_feats[t], np.float32)
    row = np.asarray(edge_index[t, 0], np.int64)
    col = np.asarray(edge_index[t, 1], np.int64)
    w = np.asarray(edge_feats[t], np.float32)

    in_maps, cfg = _preprocess(X, row, col, w)

    Wx = np.asarray(inputs["Wx"], np.float32)
    bsum = (np.asarray(inputs["bx"], np.float32)
            + np.asarray(inputs["bh"], np.float32)
            + np.asarray(inputs["bg"], np.float32)).reshape(1, -1)
    wc = np.asarray(inputs["wc"], np.float32).reshape(1, -1)
    for m in in_maps:
        m["wx_t"] = Wx
        m["bsum_t"] = bsum
        m["wc_t"] = wc
    return in_maps, cfg


def _run(inputs, trace=False):
    from concourse.bass_utils import run_bass_kernel_spmd

    in_maps, cfg = _make_in_maps(inputs)
    key = (cfg["N"], cfg["RB"], cfg["TG"],
           tuple(cfg["G"].ravel().tolist()))
    nc = _get_built(key, cfg)
    res = run_bass_kernel_spmd(nc, in_maps, core_ids=list(range(NCORES)),
                               trace=trace)
    N, R, R_PAD = cfg["N"], cfg["R"], cfg["R_PAD"]
    out = np.empty((N, P), np.float32)
    for c in range(NCORES):
        lo, hi = c * R, min((c + 1) * R, N)
        out[lo:hi] = res.results[c]["out_loc"][: hi - lo].astype(np.float32)
    return out, res.exec_time_ns


def kernel(**inputs) -> np.ndarray:
    out, _ = _run(inputs, trace=False)
    return out
